# revision 23
# baseline (speedup 1.0000x reference)
"""Multi-head attention block (B=8, N=1024, D=768, H=12 heads) on 8 trn2 NeuronCores.

Sharding: pure data-parallel over the batch dimension (one batch element per
core, weights replicated). No collectives needed.

Per-core kernel (Bass/Tile). v5:
  - all matmul operands bf16 (psum f32): FWL on stationary loads, 16-bit DVE
  - PE transposes (normal-mode matmul vs bf16 identity) with merged ScalarE
    evictions; x staged 8-deep on its own DMA queue, weights on the ScalarE
    queue
  - row-tiled score matmuls batched in kt-pairs (fewer tiling transitions)
  - attention flattened into one software-pipelined stream: AV matmuls lag S/exp
    by 2 k-tiles ACROSS (hp,qc) chunk boundaries so ScalarE never drains
  - softmax denominator broadcast via gpsimd.partition_broadcast (no DRAM
    round-trip)
  - output-projection tail on a dedicated 4-deep PSUM rotation
"""

import numpy as np

B, N, D = 8, 1024, 768
NH, HD = 12, 64
SCALE = HD ** -0.5  # 0.125
NT = N // 128       # 8 token tiles
NKT = D // 128      # 6 contraction tiles over D
NHP = NH // 2       # 6 head pairs

_STATE = {}


def _build():
    import concourse.bacc as bacc
    import concourse.bass as bass
    import concourse.mybir as mybir
    from concourse import tile
    from concourse.masks import make_identity

    f32 = mybir.dt.float32
    bf16 = mybir.dt.bfloat16
    EXP = mybir.ActivationFunctionType.Exp

    nc = bacc.Bacc(None, target_bir_lowering=False)
    x = nc.dram_tensor("x", [N, D], f32, kind="ExternalInput")
    wqkv = nc.dram_tensor("w_qkv", [D, 3 * D], f32, kind="ExternalInput")
    wproj = nc.dram_tensor("w_proj", [D, D], f32, kind="ExternalInput")
    bproj = nc.dram_tensor("b_proj", [D], f32, kind="ExternalInput")
    y = nc.dram_tensor("y", [N, D], f32, kind="ExternalOutput")

    with tile.TileContext(nc) as tc:
        with (
            tc.tile_pool(name="const", bufs=1) as const,
            tc.tile_pool(name="big", bufs=1) as big,
            tc.tile_pool(name="ystage", bufs=3) as ystage,
        ):
            ident = const.tile([128, 128], bf16)
            make_identity(nc, ident[:])
            zb = const.tile([128, 1], f32)
            nc.vector.memset(zb[:], 0.0)
            onef = const.tile([128, 1], f32)
            nc.vector.memset(onef[:], 1.0)
            # b_proj broadcast across partitions: tiny DMA to partition 0,
            # then an on-chip partition broadcast (keeps the slow zero-stride
            # DMA replication off the HBM-bound prologue window)
            bias_row = const.tile([1, D], f32)
            nc.sync.dma_start(bias_row[:], bproj[None, :])
            bias_bc = const.tile([128, D], f32)
            nc.gpsimd.partition_broadcast(bias_bc[:], bias_row[:])

            # persistent activations (all bf16)
            qkT = big.tile([128, 2 * NHP, N], bf16)       # q ftiles 0..5, k 6..11
            vban = big.tile([128, NT, NH, HD + 1], bf16)  # v natural + ones col
            outT = big.tile([128, NHP, N], bf16)          # attention out, transposed
            xT = big.tile([128, NKT, N], bf16)            # x transposed
            wv_sb = big.tile([128, NKT, D], bf16)
            wp_sb = big.tile([128, NKT, D], bf16)

            # ones columns for the denominator trick (value cols written by the
            # v-eviction copies below)
            nc.vector.tensor_copy(
                vban[:, :, :, HD:HD + 1].rearrange("p a b one -> p (a b one)"),
                onef[:, 0:1].to_broadcast((128, NT * NH)),
            )

            with (
                tc.tile_pool(name="s_ps", bufs=2, space="PSUM") as s_ps,
                tc.tile_pool(name="acc_ps", bufs=1, space="PSUM") as acc_ps,
                tc.tile_pool(name="qk_ps", bufs=1, space="PSUM") as qk_ps,
            ):
                # ---- Phase 1: transposes (PE) + v projection + q/k prologue
                #      as one dense PE stretch. x tiles stream on the Sync
                #      DMA queue; weights on the ScalarE DMA queue. ----
                with (
                    tc.tile_pool(name="stage", bufs=1) as stage,
                    tc.tile_pool(name="wstg", bufs=2) as wstg,
                ):
                    # one priority-ordered stream on the Sync queue: x tiles
                    # (PE-critical) interleaved with the v-weights, then the
                    # q/k-prologue weight tiles
                    xsts, wvsts, wtsts = [], [], []

                    # stripe the input loads across BOTH hardware DMA queues
                    # (Sync + ScalarE): each alone sustains only ~half the
                    # HBM bandwidth
                    def q(i):
                        return nc.sync if i % 2 == 0 else nc.scalar

                    def load_x(tt):
                        xst = stage.tile([128, D], f32, tag=f"xst{tt}",
                                         name=f"xst{tt}")
                        q(tt).dma_start(xst[:], x[tt * 128:(tt + 1) * 128, :])
                        xsts.append(xst)

                    def load_wv(kt):
                        wvst = wstg.tile([128, D], f32, tag=f"wvst{kt}",
                                         name=f"wvst{kt}")
                        q(kt).dma_start(
                            wvst[:], wqkv[kt * 128:(kt + 1) * 128, 2 * D:3 * D]
                        )
                        wvsts.append(wvst)

                    for tt in range(NT):
                        load_x(tt)
                    for kt in range(NKT):
                        load_wv(kt)
                    for i, ft in enumerate([0] * NKT + [NHP] * NKT):
                        kt = i % NKT
                        wt = wstg.tile([128, 128], f32, tag=f"wt{i}",
                                       name=f"wt{i}")
                        q(i).dma_start(
                            wt[:],
                            wqkv[kt * 128:(kt + 1) * 128,
                                 ft * 128:(ft + 1) * 128],
                        )
                        wtsts.append(wt)
                    for kt in range(NKT):
                        # wv casts on the (idle) GpSimd engine
                        nc.gpsimd.tensor_copy(wv_sb[:, kt, :], wvsts[kt][:])
                    xnbs = []
                    for tt in range(NT):
                        xnb = stage.tile([128, D], bf16, tag=f"xnb{tt}",
                                         name=f"xnb{tt}")
                        nc.vector.tensor_copy(xnb[:], xsts[tt][:])
                        xnbs.append(xnb)

                    def emit_v(tt):
                        psv = s_ps.tile([128, 1024], f32, tag="s", name="psv")
                        halves = (psv[:, 0:384], psv[:, 512:896])
                        for kt in range(NKT):
                            for fc in range(2):
                                nc.tensor.matmul(
                                    halves[fc],
                                    xT[:, kt, tt * 128:(tt + 1) * 128],
                                    wv_sb[:, kt, fc * 384:(fc + 1) * 384],
                                    start=(kt == 0),
                                    stop=(kt == NKT - 1),
                                )
                        for fc in range(2):
                            nc.vector.tensor_copy(
                                vban[:, tt, fc * 6:(fc + 1) * 6, 0:HD],
                                halves[fc].rearrange("p (h d) -> p h d", h=6),
                            )

                    # transpose x on the PE (normal-mode matmul against the
                    # bf16 identity); evictions are 2 wide ScalarE copies per
                    # token tile; PSUM banks alternate between the (idle) qk
                    # and acc rings so eviction overlaps the next transposes
                    for tt in range(NT):
                        pool = qk_ps if tt % 2 == 0 else acc_ps
                        tags = ("psq0", "psq1") if tt % 2 == 0 else \
                               ("acc0", "acc1")
                        pa = pool.tile([128, 512], f32, tag=tags[0], name="xpa")
                        pb = pool.tile([128, 512], f32, tag=tags[1], name="xpb")
                        for dt_ in range(NKT):
                            dst = (pa[:, dt_ * 128:(dt_ + 1) * 128] if dt_ < 4
                                   else pb[:, (dt_ - 4) * 128:(dt_ - 3) * 128])
                            nc.tensor.matmul(
                                dst,
                                xnbs[tt][:, dt_ * 128:(dt_ + 1) * 128],
                                ident[:],
                                start=True,
                                stop=True,
                            )
                        nc.scalar.copy(
                            xT[:, 0:4, tt * 128:(tt + 1) * 128],
                            pa[:].rearrange("p (a b) -> p a b", a=4),
                        )
                        nc.scalar.copy(
                            xT[:, 4:6, tt * 128:(tt + 1) * 128],
                            pb[:, 0:256].rearrange("p (a b) -> p a b", a=2),
                        )
                        if tt >= 1:
                            emit_v(tt - 1)
                    emit_v(NT - 1)

                    # q/k prologue for head pair 0: ft 0 (q) in the qk_ps
                    # banks, ft 6 (k) in the acc_ps banks; evictions on the
                    # (mostly idle) ScalarE
                    def qk_prologue_steps(ft, pool, tags, base):
                        psq0 = pool.tile([128, 512], f32, tag=tags[0],
                                         name=f"pq{ft}a")
                        psq1 = pool.tile([128, 512], f32, tag=tags[1],
                                         name=f"pq{ft}b")
                        psqs = (psq0, psq1)
                        for kt in range(NKT):
                            wtb = wstg.tile([128, 128], bf16, tag="wtb0",
                                            name="wtb0")
                            nc.vector.tensor_copy(wtb[:], wtsts[base + kt][:])
                            for qch in range(2):
                                nc.tensor.matmul(
                                    psqs[qch][:],
                                    wtb[:],
                                    xT[:, kt, qch * 512:(qch + 1) * 512],
                                    start=(kt == 0),
                                    stop=(kt == NKT - 1),
                                )
                        for qch in range(2):
                            nc.scalar.copy(
                                qkT[:, ft, qch * 512:(qch + 1) * 512],
                                psqs[qch][:],
                            )

                    qk_prologue_steps(0, qk_ps, ("psq0", "psq1"), 0)
                    qk_prologue_steps(NHP, acc_ps, ("acc0", "acc1"), NKT)

                # ---- attention: one flat software-pipelined stream over the
                #      12 (hp, qc) chunks. Per kt-pair superslot: 2 row-tiled
                #      S pairs, 2 exps, then 2 AV steps popped from a lagging
                #      queue (which crosses chunk boundaries), then 2-4
                #      interleave steps (q/k projection for the next head
                #      pair; output projection of finished token tiles). ----
                with (
                    tc.tile_pool(name="wq_pool", bufs=8) as wq_pool,
                    tc.tile_pool(name="wqb_pool", bufs=8) as wqb_pool,
                    tc.tile_pool(name="pt_pool", bufs=6) as pt_pool,
                    tc.tile_pool(name="wpstg", bufs=2) as wpstg,
                    tc.tile_pool(name="norm", bufs=2) as norm,
                ):
                    def qk_steps_for(ft):
                        """Generator of closures; each emits one PE step of the
                        qT/kT projection for feature tile ft (2 psum halves).
                        PSUM is allocated lazily inside the first step."""
                        box = {}

                        def mk_mm(kt):
                            def emit():
                                if "psqs" not in box:
                                    box["psqs"] = (
                                        qk_ps.tile([128, 512], f32, tag="psq0",
                                                   name="psq0"),
                                        qk_ps.tile([128, 512], f32, tag="psq1",
                                                   name="psq1"),
                                    )
                                wt = wq_pool.tile(
                                    [128, 128], f32, tag="wt", name="wt"
                                )
                                nc.sync.dma_start(
                                    wt[:],
                                    wqkv[kt * 128:(kt + 1) * 128,
                                         ft * 128:(ft + 1) * 128],
                                )
                                wtb = wqb_pool.tile(
                                    [128, 128], bf16, tag="wtb", name="wtb"
                                )
                                nc.vector.tensor_copy(wtb[:], wt[:])
                                for qch in range(2):
                                    nc.tensor.matmul(
                                        box["psqs"][qch][:],
                                        wtb[:],
                                        xT[:, kt, qch * 512:(qch + 1) * 512],
                                        start=(kt == 0),
                                        stop=(kt == NKT - 1),
                                    )
                            return emit

                        def mk_evict():
                            def emit():
                                for qch in range(2):
                                    nc.vector.tensor_copy(
                                        qkT[:, ft, qch * 512:(qch + 1) * 512],
                                        box["psqs"][qch][:],
                                    )
                            return emit

                        return [mk_mm(kt) for kt in range(NKT)] + [mk_evict()]

                    def proj_steps_for(tt):
                        """Output projection of token tile tt as interleavable
                        steps (borrows the idle qk PSUM slots)."""
                        box = {}

                        def mk_mm(j0):
                            def emit():
                                if "psys" not in box:
                                    box["psys"] = (
                                        qk_ps.tile([128, 512], f32, tag="psq0",
                                                   name="psy0"),
                                        qk_ps.tile([128, 512], f32, tag="psq1",
                                                   name="psy1"),
                                    )
                                for j in (j0, j0 + 1):
                                    for fc in range(2):
                                        nc.tensor.matmul(
                                            box["psys"][fc][:, 0:384],
                                            outT[:, j, tt * 128:(tt + 1) * 128],
                                            wp_sb[:, j, fc * 384:(fc + 1) * 384],
                                            start=(j == 0),
                                            stop=(j == NHP - 1),
                                        )
                            return emit

                        def mk_evict():
                            def emit():
                                for fc in range(2):
                                    yst = ystage.tile([128, 384], f32, tag="yst",
                                                      name="yst")
                                    nc.vector.tensor_add(
                                        yst[:], box["psys"][fc][:, 0:384],
                                        bias_bc[:, fc * 384:(fc + 1) * 384],
                                    )
                                    nc.sync.dma_start(
                                        y[tt * 128:(tt + 1) * 128,
                                          fc * 384:(fc + 1) * 384],
                                        yst[:],
                                    )
                            return emit

                        return [mk_mm(j0) for j0 in range(0, NHP, 2)] + [mk_evict()]

                    def make_chunk(hp, qc):
                        """State + closures for one (hp, qc) attention chunk."""
                        ctx = {"pts": []}

                        def emit_av(kt):
                            if "accs" not in ctx:
                                ctx["accs"] = (
                                    acc_ps.tile([HD + 1, 512], f32, tag="acc0",
                                                name="acc0"),
                                    acc_ps.tile([HD + 1, 512], f32, tag="acc1",
                                                name="acc1"),
                                )
                            pt = ctx["pts"][kt]
                            for h in range(2):
                                nc.tensor.matmul(
                                    ctx["accs"][h][:],
                                    vban[:, kt, hp * 2 + h, :],
                                    pt[:, h * 512:(h + 1) * 512],
                                    start=(kt == 0),
                                    stop=(kt == NT - 1),
                                )

                        def emit_s_exp(kt):
                            ssum = s_ps.tile([128, 1024], f32, tag="s",
                                             name="ssum")
                            for h in range(2):
                                ksl = qkT[h * 64:(h + 1) * 64, NHP + hp,
                                          kt * 128:(kt + 1) * 128]
                                qsl = qkT[h * 64:(h + 1) * 64, hp,
                                          qc * 512:(qc + 1) * 512]
                                nc.tensor.matmul(
                                    ssum[:, h * 512:(h + 1) * 512],
                                    ksl,
                                    qsl,
                                    start=True,
                                    stop=True,
                                )
                            return ssum

                        def emit_exp(ssum):
                            pt = pt_pool.tile([128, 1024], bf16, tag="pt",
                                              name="pt")
                            nc.scalar.activation(
                                pt[:], ssum[:], EXP, bias=zb[:], scale=SCALE
                            )
                            ctx["pts"].append(pt)

                        def finish():
                            # evict accumulators to SBUF (frees the acc ring),
                            # then normalize entirely from SBUF
                            accs = ctx["accs"]
                            asbs = (
                                norm.tile([HD + 1, 512], f32, tag="asb0",
                                          name="asb0"),
                                norm.tile([HD + 1, 512], f32, tag="asb1",
                                          name="asb1"),
                            )
                            nc.vector.tensor_copy(asbs[0][:], accs[0][:])
                            nc.vector.tensor_copy(asbs[1][:], accs[1][:])
                            # denominator rows -> partition 0 via SBUF->SBUF
                            # DMA (custom DVE recip needs base_partition 0)
                            dd = norm.tile([1, N], f32, tag="dd", name="dd")
                            for h in range(2):
                                nc.sync.dma_start(
                                    dd[0:1, h * 512:(h + 1) * 512],
                                    asbs[h][HD:HD + 1, :],
                                )
                            rr = norm.tile([1, N], f32, tag="rr", name="rr")
                            nc.vector.reciprocal_approx_fast(rr[:], dd[:])
                            for h in range(2):
                                rb = norm.tile([HD, 512], f32, tag=f"rb{h}",
                                               name=f"rb{h}")
                                nc.gpsimd.partition_broadcast(
                                    rb[:], rr[0:1, h * 512:(h + 1) * 512],
                                )
                                nc.vector.tensor_mul(
                                    outT[h * 64:(h + 1) * 64, hp,
                                         qc * 512:(qc + 1) * 512],
                                    asbs[h][0:HD, :],
                                    rb[:],
                                )

                        return ctx, emit_s_exp, emit_exp, emit_av, finish

                    av_queue = []   # lagging AV / finish closures
                    chunks = [(hp, qc) for hp in range(NHP) for qc in range(2)]
                    pendings = {}   # hp -> interleave steps
                    for hp in range(NHP):
                        steps = []
                        if hp + 1 < NHP:
                            steps = qk_steps_for(hp + 1) + qk_steps_for(
                                NHP + hp + 1)
                        pendings[hp] = steps

                    for ci, (hp, qc) in enumerate(chunks):
                        if qc == 0:
                            # W_proj k-tile for this head pair (ScalarE queue)
                            wpst = wpstg.tile([128, D], f32, tag="wpst",
                                              name="wpst")
                            nc.scalar.dma_start(
                                wpst[:], wproj[hp * 128:(hp + 1) * 128, :]
                            )
                            nc.vector.tensor_copy(wp_sb[:, hp, :], wpst[:])
                        ctx, emit_s_exp, emit_exp, emit_av, finish = \
                            make_chunk(hp, qc)
                        for kt2 in range(0, NT, 2):
                            if hp == NHP - 1 and qc == 1 and kt2 == 2:
                                # outT for tokens 0..511 is complete once the
                                # lagging finish(hp5, qc0) has been emitted
                                # (first pop of this chunk): project them here
                                for tt_ in range(4):
                                    pendings[hp] += proj_steps_for(tt_)
                            ssa = emit_s_exp(kt2)
                            ssb = emit_s_exp(kt2 + 1)
                            emit_exp(ssa)
                            emit_exp(ssb)
                            # lagging AV work (possibly from the previous
                            # chunk, including its normalize)
                            for _ in range(2):
                                if av_queue:
                                    av_queue.pop(0)()
                            av_queue.append(
                                (lambda k=kt2, f=emit_av: f(k)))
                            av_queue.append(
                                (lambda k=kt2 + 1, f=emit_av: f(k)))
                            if kt2 == NT - 2:
                                av_queue.append(finish)
                            # interleave steps for this head pair
                            for _ in range(4 if hp == NHP - 1 else 2):
                                if pendings[hp]:
                                    pendings[hp].pop(0)()
                    # drain
                    for cl in av_queue:
                        cl()
                    for hp in range(NHP):
                        for step in pendings[hp]:
                            step()

            # ---- output projection tail (token tiles 4..7) on a dedicated
            #      4-deep PSUM rotation (attention PSUM pools are closed) ----
            with tc.tile_pool(name="tail_ps", bufs=4, space="PSUM") as tail_ps:
                for tt in range(4, NT):
                    ps0 = tail_ps.tile([128, 384], f32, tag="ty0", name="ty0")
                    ps1 = tail_ps.tile([128, 384], f32, tag="ty1", name="ty1")
                    halves = (ps0, ps1)
                    for j in range(NHP):
                        for fc in range(2):
                            nc.tensor.matmul(
                                halves[fc][:],
                                outT[:, j, tt * 128:(tt + 1) * 128],
                                wp_sb[:, j, fc * 384:(fc + 1) * 384],
                                start=(j == 0),
                                stop=(j == NHP - 1),
                            )
                    for fc in range(2):
                        # fused evict + bias add
                        yst = ystage.tile([128, 384], f32, tag="yst",
                                          name="yst")
                        nc.vector.tensor_add(
                            yst[:], halves[fc][:],
                            bias_bc[:, fc * 384:(fc + 1) * 384],
                        )
                        nc.sync.dma_start(
                            y[tt * 128:(tt + 1) * 128,
                              fc * 384:(fc + 1) * 384],
                            yst[:],
                        )

    nc.compile()
    return nc


def kernel(**inputs) -> np.ndarray:
    from concourse.bass_utils import run_bass_kernel_spmd

    x = np.ascontiguousarray(np.asarray(inputs["x"], dtype=np.float32))
    wqkv = np.ascontiguousarray(np.asarray(inputs["W_qkv"], dtype=np.float32))
    wproj = np.ascontiguousarray(np.asarray(inputs["W_proj"], dtype=np.float32))
    bproj = np.ascontiguousarray(np.asarray(inputs["b_proj"], dtype=np.float32))

    if "nc" not in _STATE:
        _STATE["nc"] = _build()
    nc = _STATE["nc"]

    in_maps = [
        {"x": x[b], "w_qkv": wqkv, "w_proj": wproj, "b_proj": bproj}
        for b in range(B)
    ]
    res = run_bass_kernel_spmd(nc, in_maps, list(range(B)))
    out = np.stack([res.results[b]["y"] for b in range(B)], axis=0)
    return out.astype(np.float32)


# revision 24
# speedup vs baseline: 1.0355x; 1.0355x over previous
"""Multi-head attention block (B=8, N=1024, D=768, H=12 heads) on 8 trn2 NeuronCores.

Sharding: pure data-parallel over the batch dimension (one batch element per
core, weights replicated). No collectives needed.

Per-core kernel (Bass/Tile). v5:
  - all matmul operands bf16 (psum f32): FWL on stationary loads, 16-bit DVE
  - PE transposes (normal-mode matmul vs bf16 identity) with merged ScalarE
    evictions; x staged 8-deep on its own DMA queue, weights on the ScalarE
    queue
  - row-tiled score matmuls batched in kt-pairs (fewer tiling transitions)
  - attention flattened into one software-pipelined stream: AV matmuls lag S/exp
    by 2 k-tiles ACROSS (hp,qc) chunk boundaries so ScalarE never drains
  - softmax denominator broadcast via gpsimd.partition_broadcast (no DRAM
    round-trip)
  - output-projection tail on a dedicated 4-deep PSUM rotation
"""

import numpy as np

B, N, D = 8, 1024, 768
NH, HD = 12, 64
SCALE = HD ** -0.5  # 0.125
NT = N // 128       # 8 token tiles
NKT = D // 128      # 6 contraction tiles over D
NHP = NH // 2       # 6 head pairs

_STATE = {}


def _build():
    import concourse.bacc as bacc
    import concourse.bass as bass
    import concourse.mybir as mybir
    from concourse import tile
    from concourse.masks import make_identity

    f32 = mybir.dt.float32
    bf16 = mybir.dt.bfloat16
    EXP = mybir.ActivationFunctionType.Exp

    nc = bacc.Bacc(None, target_bir_lowering=False)
    x = nc.dram_tensor("x", [N, D], f32, kind="ExternalInput")
    wqkv = nc.dram_tensor("w_qkv", [D, 3 * D], f32, kind="ExternalInput")
    wproj = nc.dram_tensor("w_proj", [D, D], f32, kind="ExternalInput")
    bproj = nc.dram_tensor("b_proj", [D], f32, kind="ExternalInput")
    y = nc.dram_tensor("y", [N, D], f32, kind="ExternalOutput")

    with tile.TileContext(nc) as tc:
        with (
            tc.tile_pool(name="const", bufs=1) as const,
            tc.tile_pool(name="big", bufs=1) as big,
            tc.tile_pool(name="ystage", bufs=3) as ystage,
        ):
            ident = const.tile([128, 128], bf16)
            make_identity(nc, ident[:])
            zb = const.tile([128, 1], f32)
            nc.vector.memset(zb[:], 0.0)
            onef = const.tile([128, 1], f32)
            nc.vector.memset(onef[:], 1.0)
            # b_proj broadcast across partitions: tiny DMA to partition 0,
            # then an on-chip partition broadcast (keeps the slow zero-stride
            # DMA replication off the HBM-bound prologue window)
            bias_row = const.tile([1, D], f32)
            nc.sync.dma_start(bias_row[:], bproj[None, :])
            bias_bc = const.tile([128, D], f32)
            nc.gpsimd.partition_broadcast(bias_bc[:], bias_row[:])

            # persistent activations (all bf16)
            qkT = big.tile([128, 2 * NHP, N], bf16)       # q ftiles 0..5, k 6..11
            vban = big.tile([128, NT, NH, HD + 1], bf16)  # v natural + ones col
            outT = big.tile([128, NHP, N], bf16)          # attention out, transposed
            xT = big.tile([128, NKT, N], bf16)            # x transposed
            wv_sb = big.tile([128, NKT, D], bf16)
            wp_sb = big.tile([128, NKT, D], bf16)

            # ones columns for the denominator trick (value cols written by the
            # v-eviction copies below)
            nc.vector.tensor_copy(
                vban[:, :, :, HD:HD + 1].rearrange("p a b one -> p (a b one)"),
                onef[:, 0:1].to_broadcast((128, NT * NH)),
            )

            with (
                tc.tile_pool(name="s_ps", bufs=2, space="PSUM") as s_ps,
                tc.tile_pool(name="acc_ps", bufs=1, space="PSUM") as acc_ps,
                tc.tile_pool(name="qk_ps", bufs=1, space="PSUM") as qk_ps,
            ):
                # ---- Phase 1: transposes (PE) + v projection + q/k prologue
                #      as one dense PE stretch. x tiles stream on the Sync
                #      DMA queue; weights on the ScalarE DMA queue. ----
                with (
                    tc.tile_pool(name="stage", bufs=1) as stage,
                    tc.tile_pool(name="wstg", bufs=2) as wstg,
                ):
                    # one priority-ordered stream on the Sync queue: x tiles
                    # (PE-critical) interleaved with the v-weights, then the
                    # q/k-prologue weight tiles
                    xsts, wvsts, wtsts = [], [], []

                    # stripe the input loads across BOTH hardware DMA queues
                    # (Sync + ScalarE): each alone sustains only ~half the
                    # HBM bandwidth
                    def q(i):
                        return nc.sync if i % 2 == 0 else nc.scalar

                    def load_x(tt):
                        xst = stage.tile([128, D], f32, tag=f"xst{tt}",
                                         name=f"xst{tt}")
                        q(tt).dma_start(xst[:], x[tt * 128:(tt + 1) * 128, :])
                        xsts.append(xst)

                    def load_wv(kt):
                        wvst = wstg.tile([128, D], f32, tag=f"wvst{kt}",
                                         name=f"wvst{kt}")
                        q(kt).dma_start(
                            wvst[:], wqkv[kt * 128:(kt + 1) * 128, 2 * D:3 * D]
                        )
                        wvsts.append(wvst)

                    for tt in range(NT):
                        load_x(tt)
                    for kt in range(NKT):
                        load_wv(kt)
                    for i, ft in enumerate([0] * NKT + [NHP] * NKT):
                        kt = i % NKT
                        wt = wstg.tile([128, 128], f32, tag=f"wt{i}",
                                       name=f"wt{i}")
                        q(i).dma_start(
                            wt[:],
                            wqkv[kt * 128:(kt + 1) * 128,
                                 ft * 128:(ft + 1) * 128],
                        )
                        wtsts.append(wt)
                    xnbs = []
                    for tt in range(NT):
                        xnb = stage.tile([128, D], bf16, tag=f"xnb{tt}",
                                         name=f"xnb{tt}")
                        nc.vector.tensor_copy(xnb[:], xsts[tt][:])
                        xnbs.append(xnb)
                        if tt < NKT:
                            nc.vector.tensor_copy(wv_sb[:, tt, :],
                                                  wvsts[tt][:])

                    def emit_v(tt):
                        psv = s_ps.tile([128, 1024], f32, tag="s", name="psv")
                        halves = (psv[:, 0:384], psv[:, 512:896])
                        for kt in range(NKT):
                            for fc in range(2):
                                nc.tensor.matmul(
                                    halves[fc],
                                    xT[:, kt, tt * 128:(tt + 1) * 128],
                                    wv_sb[:, kt, fc * 384:(fc + 1) * 384],
                                    start=(kt == 0),
                                    stop=(kt == NKT - 1),
                                )
                        for fc in range(2):
                            nc.vector.tensor_copy(
                                vban[:, tt, fc * 6:(fc + 1) * 6, 0:HD],
                                halves[fc].rearrange("p (h d) -> p h d", h=6),
                            )

                    # transpose x on the PE (normal-mode matmul against the
                    # bf16 identity); evictions are 2 wide ScalarE copies per
                    # token tile; PSUM banks alternate between the (idle) qk
                    # and acc rings so eviction overlaps the next transposes
                    for tt in range(NT):
                        pool = qk_ps if tt % 2 == 0 else acc_ps
                        tags = ("psq0", "psq1") if tt % 2 == 0 else \
                               ("acc0", "acc1")
                        pa = pool.tile([128, 512], f32, tag=tags[0], name="xpa")
                        pb = pool.tile([128, 512], f32, tag=tags[1], name="xpb")
                        for dt_ in range(NKT):
                            dst = (pa[:, dt_ * 128:(dt_ + 1) * 128] if dt_ < 4
                                   else pb[:, (dt_ - 4) * 128:(dt_ - 3) * 128])
                            nc.tensor.matmul(
                                dst,
                                xnbs[tt][:, dt_ * 128:(dt_ + 1) * 128],
                                ident[:],
                                start=True,
                                stop=True,
                            )
                        nc.scalar.copy(
                            xT[:, 0:4, tt * 128:(tt + 1) * 128],
                            pa[:].rearrange("p (a b) -> p a b", a=4),
                        )
                        nc.scalar.copy(
                            xT[:, 4:6, tt * 128:(tt + 1) * 128],
                            pb[:, 0:256].rearrange("p (a b) -> p a b", a=2),
                        )
                        if tt >= 1:
                            emit_v(tt - 1)
                    emit_v(NT - 1)

                    # q/k prologue for head pair 0: ft 0 (q) in the qk_ps
                    # banks, ft 6 (k) in the acc_ps banks; evictions on the
                    # (mostly idle) ScalarE
                    def qk_prologue_steps(ft, pool, tags, base):
                        psq0 = pool.tile([128, 512], f32, tag=tags[0],
                                         name=f"pq{ft}a")
                        psq1 = pool.tile([128, 512], f32, tag=tags[1],
                                         name=f"pq{ft}b")
                        psqs = (psq0, psq1)
                        for kt in range(NKT):
                            wtb = wstg.tile([128, 128], bf16, tag="wtb0",
                                            name="wtb0")
                            nc.vector.tensor_copy(wtb[:], wtsts[base + kt][:])
                            for qch in range(2):
                                nc.tensor.matmul(
                                    psqs[qch][:],
                                    wtb[:],
                                    xT[:, kt, qch * 512:(qch + 1) * 512],
                                    start=(kt == 0),
                                    stop=(kt == NKT - 1),
                                )
                        for qch in range(2):
                            nc.scalar.copy(
                                qkT[:, ft, qch * 512:(qch + 1) * 512],
                                psqs[qch][:],
                            )

                    qk_prologue_steps(0, qk_ps, ("psq0", "psq1"), 0)
                    qk_prologue_steps(NHP, acc_ps, ("acc0", "acc1"), NKT)

                # ---- attention: one flat software-pipelined stream over the
                #      12 (hp, qc) chunks. Per kt-pair superslot: 2 row-tiled
                #      S pairs, 2 exps, then 2 AV steps popped from a lagging
                #      queue (which crosses chunk boundaries), then 2-4
                #      interleave steps (q/k projection for the next head
                #      pair; output projection of finished token tiles). ----
                with (
                    tc.tile_pool(name="wq_pool", bufs=8) as wq_pool,
                    tc.tile_pool(name="wqb_pool", bufs=8) as wqb_pool,
                    tc.tile_pool(name="pt_pool", bufs=6) as pt_pool,
                    tc.tile_pool(name="wpstg", bufs=2) as wpstg,
                    tc.tile_pool(name="norm", bufs=2) as norm,
                ):
                    def qk_steps_for(ft):
                        """Generator of closures; each emits one PE step of the
                        qT/kT projection for feature tile ft (2 psum halves).
                        PSUM is allocated lazily inside the first step."""
                        box = {}

                        def mk_mm(kt):
                            def emit():
                                if "psqs" not in box:
                                    box["psqs"] = (
                                        qk_ps.tile([128, 512], f32, tag="psq0",
                                                   name="psq0"),
                                        qk_ps.tile([128, 512], f32, tag="psq1",
                                                   name="psq1"),
                                    )
                                wt = wq_pool.tile(
                                    [128, 128], f32, tag="wt", name="wt"
                                )
                                nc.sync.dma_start(
                                    wt[:],
                                    wqkv[kt * 128:(kt + 1) * 128,
                                         ft * 128:(ft + 1) * 128],
                                )
                                wtb = wqb_pool.tile(
                                    [128, 128], bf16, tag="wtb", name="wtb"
                                )
                                nc.vector.tensor_copy(wtb[:], wt[:])
                                for qch in range(2):
                                    nc.tensor.matmul(
                                        box["psqs"][qch][:],
                                        wtb[:],
                                        xT[:, kt, qch * 512:(qch + 1) * 512],
                                        start=(kt == 0),
                                        stop=(kt == NKT - 1),
                                    )
                            return emit

                        def mk_evict():
                            def emit():
                                for qch in range(2):
                                    nc.vector.tensor_copy(
                                        qkT[:, ft, qch * 512:(qch + 1) * 512],
                                        box["psqs"][qch][:],
                                    )
                            return emit

                        return [mk_mm(kt) for kt in range(NKT)] + [mk_evict()]

                    def proj_steps_for(tt):
                        """Output projection of token tile tt as interleavable
                        steps (borrows the idle qk PSUM slots)."""
                        box = {}

                        def mk_mm(j0):
                            def emit():
                                if "psys" not in box:
                                    box["psys"] = (
                                        qk_ps.tile([128, 512], f32, tag="psq0",
                                                   name="psy0"),
                                        qk_ps.tile([128, 512], f32, tag="psq1",
                                                   name="psy1"),
                                    )
                                for j in (j0, j0 + 1):
                                    for fc in range(2):
                                        nc.tensor.matmul(
                                            box["psys"][fc][:, 0:384],
                                            outT[:, j, tt * 128:(tt + 1) * 128],
                                            wp_sb[:, j, fc * 384:(fc + 1) * 384],
                                            start=(j == 0),
                                            stop=(j == NHP - 1),
                                        )
                            return emit

                        def mk_evict():
                            def emit():
                                for fc in range(2):
                                    yst = ystage.tile([128, 384], f32, tag="yst",
                                                      name="yst")
                                    nc.vector.tensor_add(
                                        yst[:], box["psys"][fc][:, 0:384],
                                        bias_bc[:, fc * 384:(fc + 1) * 384],
                                    )
                                    nc.sync.dma_start(
                                        y[tt * 128:(tt + 1) * 128,
                                          fc * 384:(fc + 1) * 384],
                                        yst[:],
                                    )
                            return emit

                        return [mk_mm(j0) for j0 in range(0, NHP, 2)] + [mk_evict()]

                    def make_chunk(hp, qc):
                        """State + closures for one (hp, qc) attention chunk."""
                        ctx = {"pts": []}

                        def emit_av(kt):
                            if "accs" not in ctx:
                                ctx["accs"] = (
                                    acc_ps.tile([HD + 1, 512], f32, tag="acc0",
                                                name="acc0"),
                                    acc_ps.tile([HD + 1, 512], f32, tag="acc1",
                                                name="acc1"),
                                )
                            pt = ctx["pts"][kt]
                            for h in range(2):
                                nc.tensor.matmul(
                                    ctx["accs"][h][:],
                                    vban[:, kt, hp * 2 + h, :],
                                    pt[:, h * 512:(h + 1) * 512],
                                    start=(kt == 0),
                                    stop=(kt == NT - 1),
                                )

                        def emit_s_exp(kt):
                            ssum = s_ps.tile([128, 1024], f32, tag="s",
                                             name="ssum")
                            for h in range(2):
                                ksl = qkT[h * 64:(h + 1) * 64, NHP + hp,
                                          kt * 128:(kt + 1) * 128]
                                qsl = qkT[h * 64:(h + 1) * 64, hp,
                                          qc * 512:(qc + 1) * 512]
                                nc.tensor.matmul(
                                    ssum[:, h * 512:(h + 1) * 512],
                                    ksl,
                                    qsl,
                                    start=True,
                                    stop=True,
                                )
                            return ssum

                        def emit_exp(ssum):
                            pt = pt_pool.tile([128, 1024], bf16, tag="pt",
                                              name="pt")
                            nc.scalar.activation(
                                pt[:], ssum[:], EXP, bias=zb[:], scale=SCALE
                            )
                            ctx["pts"].append(pt)

                        def finish():
                            # evict accumulators to SBUF (frees the acc ring),
                            # then normalize entirely from SBUF
                            accs = ctx["accs"]
                            asbs = (
                                norm.tile([HD + 1, 512], f32, tag="asb0",
                                          name="asb0"),
                                norm.tile([HD + 1, 512], f32, tag="asb1",
                                          name="asb1"),
                            )
                            nc.vector.tensor_copy(asbs[0][:], accs[0][:])
                            nc.vector.tensor_copy(asbs[1][:], accs[1][:])
                            # denominator rows -> partition 0 via SBUF->SBUF
                            # DMA (custom DVE recip needs base_partition 0)
                            dd = norm.tile([1, N], f32, tag="dd", name="dd")
                            for h in range(2):
                                nc.sync.dma_start(
                                    dd[0:1, h * 512:(h + 1) * 512],
                                    asbs[h][HD:HD + 1, :],
                                )
                            rr = norm.tile([1, N], f32, tag="rr", name="rr")
                            nc.vector.reciprocal_approx_fast(rr[:], dd[:])
                            for h in range(2):
                                rb = norm.tile([HD, 512], f32, tag=f"rb{h}",
                                               name=f"rb{h}")
                                nc.gpsimd.partition_broadcast(
                                    rb[:], rr[0:1, h * 512:(h + 1) * 512],
                                )
                                nc.vector.tensor_mul(
                                    outT[h * 64:(h + 1) * 64, hp,
                                         qc * 512:(qc + 1) * 512],
                                    asbs[h][0:HD, :],
                                    rb[:],
                                )

                        return ctx, emit_s_exp, emit_exp, emit_av, finish

                    av_queue = []   # lagging AV / finish closures
                    chunks = [(hp, qc) for hp in range(NHP) for qc in range(2)]
                    pendings = {}   # hp -> interleave steps
                    for hp in range(NHP):
                        steps = []
                        if hp + 1 < NHP:
                            steps = qk_steps_for(hp + 1) + qk_steps_for(
                                NHP + hp + 1)
                        pendings[hp] = steps

                    for ci, (hp, qc) in enumerate(chunks):
                        if qc == 0:
                            # W_proj k-tile for this head pair (ScalarE queue)
                            wpst = wpstg.tile([128, D], f32, tag="wpst",
                                              name="wpst")
                            nc.scalar.dma_start(
                                wpst[:], wproj[hp * 128:(hp + 1) * 128, :]
                            )
                            nc.vector.tensor_copy(wp_sb[:, hp, :], wpst[:])
                        ctx, emit_s_exp, emit_exp, emit_av, finish = \
                            make_chunk(hp, qc)
                        for kt2 in range(0, NT, 2):
                            if hp == NHP - 1 and qc == 1 and kt2 == 2:
                                # outT for tokens 0..511 is complete once the
                                # lagging finish(hp5, qc0) has been emitted
                                # (first pop of this chunk): project them here
                                for tt_ in range(4):
                                    pendings[hp] += proj_steps_for(tt_)
                            ssa = emit_s_exp(kt2)
                            ssb = emit_s_exp(kt2 + 1)
                            emit_exp(ssa)
                            emit_exp(ssb)
                            # lagging AV work (possibly from the previous
                            # chunk, including its normalize)
                            for _ in range(2):
                                if av_queue:
                                    av_queue.pop(0)()
                            av_queue.append(
                                (lambda k=kt2, f=emit_av: f(k)))
                            av_queue.append(
                                (lambda k=kt2 + 1, f=emit_av: f(k)))
                            if kt2 == NT - 2:
                                av_queue.append(finish)
                            # interleave steps for this head pair
                            for _ in range(4 if hp == NHP - 1 else 2):
                                if pendings[hp]:
                                    pendings[hp].pop(0)()
                    # drain
                    for cl in av_queue:
                        cl()
                    for hp in range(NHP):
                        for step in pendings[hp]:
                            step()

            # ---- output projection tail (token tiles 4..7) on a dedicated
            #      4-deep PSUM rotation (attention PSUM pools are closed) ----
            with tc.tile_pool(name="tail_ps", bufs=4, space="PSUM") as tail_ps:
                for tt in range(4, NT):
                    ps0 = tail_ps.tile([128, 384], f32, tag="ty0", name="ty0")
                    ps1 = tail_ps.tile([128, 384], f32, tag="ty1", name="ty1")
                    halves = (ps0, ps1)
                    for j in range(NHP):
                        for fc in range(2):
                            nc.tensor.matmul(
                                halves[fc][:],
                                outT[:, j, tt * 128:(tt + 1) * 128],
                                wp_sb[:, j, fc * 384:(fc + 1) * 384],
                                start=(j == 0),
                                stop=(j == NHP - 1),
                            )
                    for fc in range(2):
                        # fused evict + bias add
                        yst = ystage.tile([128, 384], f32, tag="yst",
                                          name="yst")
                        nc.vector.tensor_add(
                            yst[:], halves[fc][:],
                            bias_bc[:, fc * 384:(fc + 1) * 384],
                        )
                        nc.sync.dma_start(
                            y[tt * 128:(tt + 1) * 128,
                              fc * 384:(fc + 1) * 384],
                            yst[:],
                        )

    nc.compile()
    return nc


def kernel(**inputs) -> np.ndarray:
    from concourse.bass_utils import run_bass_kernel_spmd

    x = np.ascontiguousarray(np.asarray(inputs["x"], dtype=np.float32))
    wqkv = np.ascontiguousarray(np.asarray(inputs["W_qkv"], dtype=np.float32))
    wproj = np.ascontiguousarray(np.asarray(inputs["W_proj"], dtype=np.float32))
    bproj = np.ascontiguousarray(np.asarray(inputs["b_proj"], dtype=np.float32))

    if "nc" not in _STATE:
        _STATE["nc"] = _build()
    nc = _STATE["nc"]

    in_maps = [
        {"x": x[b], "w_qkv": wqkv, "w_proj": wproj, "b_proj": bproj}
        for b in range(B)
    ]
    res = run_bass_kernel_spmd(nc, in_maps, list(range(B)))
    out = np.stack([res.results[b]["y"] for b in range(B)], axis=0)
    return out.astype(np.float32)


# revision 25
# speedup vs baseline: 1.0515x; 1.0155x over previous
"""Multi-head attention block (B=8, N=1024, D=768, H=12 heads) on 8 trn2 NeuronCores.

Sharding: pure data-parallel over the batch dimension (one batch element per
core, weights replicated). No collectives needed.

Per-core kernel (Bass/Tile). v5:
  - all matmul operands bf16 (psum f32): FWL on stationary loads, 16-bit DVE
  - PE transposes (normal-mode matmul vs bf16 identity) with merged ScalarE
    evictions; x staged 8-deep on its own DMA queue, weights on the ScalarE
    queue
  - row-tiled score matmuls batched in kt-pairs (fewer tiling transitions)
  - attention flattened into one software-pipelined stream: AV matmuls lag S/exp
    by 2 k-tiles ACROSS (hp,qc) chunk boundaries so ScalarE never drains
  - softmax denominator broadcast via gpsimd.partition_broadcast (no DRAM
    round-trip)
  - output-projection tail on a dedicated 4-deep PSUM rotation
"""

import numpy as np

B, N, D = 8, 1024, 768
NH, HD = 12, 64
SCALE = HD ** -0.5  # 0.125
NT = N // 128       # 8 token tiles
NKT = D // 128      # 6 contraction tiles over D
NHP = NH // 2       # 6 head pairs

_STATE = {}


def _build():
    import concourse.bacc as bacc
    import concourse.bass as bass
    import concourse.mybir as mybir
    from concourse import tile
    from concourse.masks import make_identity

    f32 = mybir.dt.float32
    bf16 = mybir.dt.bfloat16
    EXP = mybir.ActivationFunctionType.Exp

    nc = bacc.Bacc(None, target_bir_lowering=False)
    x = nc.dram_tensor("x", [N, D], f32, kind="ExternalInput")
    wqkv = nc.dram_tensor("w_qkv", [D, 3 * D], f32, kind="ExternalInput")
    wproj = nc.dram_tensor("w_proj", [D, D], f32, kind="ExternalInput")
    bproj = nc.dram_tensor("b_proj", [D], f32, kind="ExternalInput")
    y = nc.dram_tensor("y", [N, D], f32, kind="ExternalOutput")

    with tile.TileContext(nc) as tc:
        with (
            tc.tile_pool(name="const", bufs=1) as const,
            tc.tile_pool(name="big", bufs=1) as big,
            tc.tile_pool(name="ystage", bufs=3) as ystage,
        ):
            ident = const.tile([128, 128], bf16)
            make_identity(nc, ident[:])
            zb = const.tile([128, 1], f32)
            nc.vector.memset(zb[:], 0.0)
            onef = const.tile([128, 1], f32)
            nc.vector.memset(onef[:], 1.0)
            # b_proj broadcast across partitions: tiny DMA to partition 0,
            # then an on-chip partition broadcast (keeps the slow zero-stride
            # DMA replication off the HBM-bound prologue window)
            bias_row = const.tile([1, D], f32)
            nc.sync.dma_start(bias_row[:], bproj[None, :])
            bias_bc = const.tile([128, D], f32)
            nc.gpsimd.partition_broadcast(bias_bc[:], bias_row[:])

            # persistent activations (all bf16)
            qkT = big.tile([128, 2 * NHP, N], bf16)       # q ftiles 0..5, k 6..11
            vban = big.tile([128, NT, NH, HD + 1], bf16)  # v natural + ones col
            outT = big.tile([128, NHP, N], bf16)          # attention out, transposed
            xT = big.tile([128, NKT, N], bf16)            # x transposed
            wv_sb = big.tile([128, NKT, D], bf16)
            wp_sb = big.tile([128, NKT, D], bf16)

            # ones columns for the denominator trick (value cols written by the
            # v-eviction copies below)
            nc.vector.tensor_copy(
                vban[:, :, :, HD:HD + 1].rearrange("p a b one -> p (a b one)"),
                onef[:, 0:1].to_broadcast((128, NT * NH)),
            )

            with (
                tc.tile_pool(name="s_ps", bufs=2, space="PSUM") as s_ps,
                tc.tile_pool(name="acc_ps", bufs=1, space="PSUM") as acc_ps,
                tc.tile_pool(name="qk_ps", bufs=1, space="PSUM") as qk_ps,
            ):
                # ---- Phase 1: transposes (PE) + v projection + q/k prologue
                #      as one dense PE stretch. x tiles stream on the Sync
                #      DMA queue; weights on the ScalarE DMA queue. ----
                with (
                    tc.tile_pool(name="stage", bufs=1) as stage,
                    tc.tile_pool(name="wstg", bufs=2) as wstg,
                ):
                    # one priority-ordered stream on the Sync queue: x tiles
                    # (PE-critical) interleaved with the v-weights, then the
                    # q/k-prologue weight tiles
                    xsts, wvsts, wtsts = [], [], []

                    def load_x(tt):
                        xst = stage.tile([128, D], f32, tag=f"xst{tt}",
                                         name=f"xst{tt}")
                        nc.sync.dma_start(xst[:], x[tt * 128:(tt + 1) * 128, :])
                        xsts.append(xst)

                    def load_wv(kt):
                        wvst = wstg.tile([128, D], f32, tag=f"wvst{kt}",
                                         name=f"wvst{kt}")
                        nc.sync.dma_start(
                            wvst[:], wqkv[kt * 128:(kt + 1) * 128, 2 * D:3 * D]
                        )
                        wvsts.append(wvst)

                    for tt in range(3):
                        load_x(tt)
                    for kt in range(NKT):
                        load_wv(kt)
                        if kt + 3 < NT:
                            load_x(kt + 3)
                    for i, ft in enumerate([0] * NKT + [NHP] * NKT):
                        kt = i % NKT
                        wt = wstg.tile([128, 128], f32, tag=f"wt{i}",
                                       name=f"wt{i}")
                        nc.sync.dma_start(
                            wt[:],
                            wqkv[kt * 128:(kt + 1) * 128,
                                 ft * 128:(ft + 1) * 128],
                        )
                        wtsts.append(wt)
                    xnbs = []
                    for tt in range(NT):
                        xnb = stage.tile([128, D], bf16, tag=f"xnb{tt}",
                                         name=f"xnb{tt}")
                        nc.vector.tensor_copy(xnb[:], xsts[tt][:])
                        xnbs.append(xnb)
                        if tt < NKT:
                            nc.vector.tensor_copy(wv_sb[:, tt, :],
                                                  wvsts[tt][:])

                    def emit_v(tt):
                        psv = s_ps.tile([128, 1024], f32, tag="s", name="psv")
                        halves = (psv[:, 0:384], psv[:, 512:896])
                        for kt in range(NKT):
                            for fc in range(2):
                                nc.tensor.matmul(
                                    halves[fc],
                                    xT[:, kt, tt * 128:(tt + 1) * 128],
                                    wv_sb[:, kt, fc * 384:(fc + 1) * 384],
                                    start=(kt == 0),
                                    stop=(kt == NKT - 1),
                                )
                        for fc in range(2):
                            nc.vector.tensor_copy(
                                vban[:, tt, fc * 6:(fc + 1) * 6, 0:HD],
                                halves[fc].rearrange("p (h d) -> p h d", h=6),
                            )

                    # transpose x on the PE (normal-mode matmul against the
                    # bf16 identity); evictions are 2 wide ScalarE copies per
                    # token tile; PSUM banks alternate between the (idle) qk
                    # and acc rings so eviction overlaps the next transposes
                    for tt in range(NT):
                        pool = qk_ps if tt % 2 == 0 else acc_ps
                        tags = ("psq0", "psq1") if tt % 2 == 0 else \
                               ("acc0", "acc1")
                        pa = pool.tile([128, 512], f32, tag=tags[0], name="xpa")
                        pb = pool.tile([128, 512], f32, tag=tags[1], name="xpb")
                        for dt_ in range(NKT):
                            dst = (pa[:, dt_ * 128:(dt_ + 1) * 128] if dt_ < 4
                                   else pb[:, (dt_ - 4) * 128:(dt_ - 3) * 128])
                            nc.tensor.matmul(
                                dst,
                                xnbs[tt][:, dt_ * 128:(dt_ + 1) * 128],
                                ident[:],
                                start=True,
                                stop=True,
                            )
                        nc.scalar.copy(
                            xT[:, 0:4, tt * 128:(tt + 1) * 128],
                            pa[:].rearrange("p (a b) -> p a b", a=4),
                        )
                        nc.scalar.copy(
                            xT[:, 4:6, tt * 128:(tt + 1) * 128],
                            pb[:, 0:256].rearrange("p (a b) -> p a b", a=2),
                        )
                        if tt >= 1:
                            emit_v(tt - 1)
                    emit_v(NT - 1)

                    # q/k prologue for head pair 0: ft 0 (q) in the qk_ps
                    # banks, ft 6 (k) in the acc_ps banks; evictions on the
                    # (mostly idle) ScalarE
                    def qk_prologue_steps(ft, pool, tags, base):
                        psq0 = pool.tile([128, 512], f32, tag=tags[0],
                                         name=f"pq{ft}a")
                        psq1 = pool.tile([128, 512], f32, tag=tags[1],
                                         name=f"pq{ft}b")
                        psqs = (psq0, psq1)
                        for kt in range(NKT):
                            wtb = wstg.tile([128, 128], bf16, tag="wtb0",
                                            name="wtb0")
                            nc.vector.tensor_copy(wtb[:], wtsts[base + kt][:])
                            for qch in range(2):
                                nc.tensor.matmul(
                                    psqs[qch][:],
                                    wtb[:],
                                    xT[:, kt, qch * 512:(qch + 1) * 512],
                                    start=(kt == 0),
                                    stop=(kt == NKT - 1),
                                )
                        for qch in range(2):
                            nc.scalar.copy(
                                qkT[:, ft, qch * 512:(qch + 1) * 512],
                                psqs[qch][:],
                            )

                    qk_prologue_steps(0, qk_ps, ("psq0", "psq1"), 0)
                    qk_prologue_steps(NHP, acc_ps, ("acc0", "acc1"), NKT)

                # ---- attention: one flat software-pipelined stream over the
                #      12 (hp, qc) chunks. Per kt-pair superslot: 2 row-tiled
                #      S pairs, 2 exps, then 2 AV steps popped from a lagging
                #      queue (which crosses chunk boundaries), then 2-4
                #      interleave steps (q/k projection for the next head
                #      pair; output projection of finished token tiles). ----
                with (
                    tc.tile_pool(name="wq_pool", bufs=8) as wq_pool,
                    tc.tile_pool(name="wqb_pool", bufs=8) as wqb_pool,
                    tc.tile_pool(name="pt_pool", bufs=6) as pt_pool,
                    tc.tile_pool(name="wpstg", bufs=2) as wpstg,
                    tc.tile_pool(name="norm", bufs=2) as norm,
                ):
                    def qk_steps_for(ft):
                        """Generator of closures; each emits one PE step of the
                        qT/kT projection for feature tile ft (2 psum halves).
                        PSUM is allocated lazily inside the first step."""
                        box = {}

                        def mk_mm(kt):
                            def emit():
                                if "psqs" not in box:
                                    box["psqs"] = (
                                        qk_ps.tile([128, 512], f32, tag="psq0",
                                                   name="psq0"),
                                        qk_ps.tile([128, 512], f32, tag="psq1",
                                                   name="psq1"),
                                    )
                                wt = wq_pool.tile(
                                    [128, 128], f32, tag="wt", name="wt"
                                )
                                nc.sync.dma_start(
                                    wt[:],
                                    wqkv[kt * 128:(kt + 1) * 128,
                                         ft * 128:(ft + 1) * 128],
                                )
                                wtb = wqb_pool.tile(
                                    [128, 128], bf16, tag="wtb", name="wtb"
                                )
                                nc.vector.tensor_copy(wtb[:], wt[:])
                                for qch in range(2):
                                    nc.tensor.matmul(
                                        box["psqs"][qch][:],
                                        wtb[:],
                                        xT[:, kt, qch * 512:(qch + 1) * 512],
                                        start=(kt == 0),
                                        stop=(kt == NKT - 1),
                                    )
                            return emit

                        def mk_evict():
                            def emit():
                                for qch in range(2):
                                    nc.vector.tensor_copy(
                                        qkT[:, ft, qch * 512:(qch + 1) * 512],
                                        box["psqs"][qch][:],
                                    )
                            return emit

                        return [mk_mm(kt) for kt in range(NKT)] + [mk_evict()]

                    def proj_steps_for(tt):
                        """Output projection of token tile tt as interleavable
                        steps (borrows the idle qk PSUM slots)."""
                        box = {}

                        def mk_mm(j0):
                            def emit():
                                if "psys" not in box:
                                    box["psys"] = (
                                        qk_ps.tile([128, 512], f32, tag="psq0",
                                                   name="psy0"),
                                        qk_ps.tile([128, 512], f32, tag="psq1",
                                                   name="psy1"),
                                    )
                                for j in (j0, j0 + 1):
                                    for fc in range(2):
                                        nc.tensor.matmul(
                                            box["psys"][fc][:, 0:384],
                                            outT[:, j, tt * 128:(tt + 1) * 128],
                                            wp_sb[:, j, fc * 384:(fc + 1) * 384],
                                            start=(j == 0),
                                            stop=(j == NHP - 1),
                                        )
                            return emit

                        def mk_evict():
                            def emit():
                                for fc in range(2):
                                    yst = ystage.tile([128, 384], f32, tag="yst",
                                                      name="yst")
                                    nc.vector.tensor_add(
                                        yst[:], box["psys"][fc][:, 0:384],
                                        bias_bc[:, fc * 384:(fc + 1) * 384],
                                    )
                                    nc.sync.dma_start(
                                        y[tt * 128:(tt + 1) * 128,
                                          fc * 384:(fc + 1) * 384],
                                        yst[:],
                                    )
                            return emit

                        return [mk_mm(j0) for j0 in range(0, NHP, 2)] + [mk_evict()]

                    def make_chunk(hp, qc):
                        """State + closures for one (hp, qc) attention chunk."""
                        ctx = {"pts": []}

                        def emit_av(kt):
                            if "accs" not in ctx:
                                ctx["accs"] = (
                                    acc_ps.tile([HD + 1, 512], f32, tag="acc0",
                                                name="acc0"),
                                    acc_ps.tile([HD + 1, 512], f32, tag="acc1",
                                                name="acc1"),
                                )
                            pt = ctx["pts"][kt]
                            for h in range(2):
                                nc.tensor.matmul(
                                    ctx["accs"][h][:],
                                    vban[:, kt, hp * 2 + h, :],
                                    pt[:, h * 512:(h + 1) * 512],
                                    start=(kt == 0),
                                    stop=(kt == NT - 1),
                                )

                        def emit_s_exp(kt):
                            ssum = s_ps.tile([128, 1024], f32, tag="s",
                                             name="ssum")
                            for h in range(2):
                                ksl = qkT[h * 64:(h + 1) * 64, NHP + hp,
                                          kt * 128:(kt + 1) * 128]
                                qsl = qkT[h * 64:(h + 1) * 64, hp,
                                          qc * 512:(qc + 1) * 512]
                                nc.tensor.matmul(
                                    ssum[:, h * 512:(h + 1) * 512],
                                    ksl,
                                    qsl,
                                    start=True,
                                    stop=True,
                                )
                            return ssum

                        def emit_exp(ssum):
                            pt = pt_pool.tile([128, 1024], bf16, tag="pt",
                                              name="pt")
                            nc.scalar.activation(
                                pt[:], ssum[:], EXP, bias=zb[:], scale=SCALE
                            )
                            ctx["pts"].append(pt)

                        def finish():
                            # evict accumulators to SBUF (frees the acc ring),
                            # then normalize entirely from SBUF
                            accs = ctx["accs"]
                            asbs = (
                                norm.tile([HD + 1, 512], f32, tag="asb0",
                                          name="asb0"),
                                norm.tile([HD + 1, 512], f32, tag="asb1",
                                          name="asb1"),
                            )
                            nc.vector.tensor_copy(asbs[0][:], accs[0][:])
                            nc.vector.tensor_copy(asbs[1][:], accs[1][:])
                            # denominator rows -> partition 0 via SBUF->SBUF
                            # DMA (custom DVE recip needs base_partition 0)
                            dd = norm.tile([1, N], f32, tag="dd", name="dd")
                            for h in range(2):
                                nc.sync.dma_start(
                                    dd[0:1, h * 512:(h + 1) * 512],
                                    asbs[h][HD:HD + 1, :],
                                )
                            rr = norm.tile([1, N], f32, tag="rr", name="rr")
                            nc.vector.reciprocal_approx_fast(rr[:], dd[:])
                            for h in range(2):
                                rb = norm.tile([HD, 512], f32, tag=f"rb{h}",
                                               name=f"rb{h}")
                                nc.gpsimd.partition_broadcast(
                                    rb[:], rr[0:1, h * 512:(h + 1) * 512],
                                )
                                nc.vector.tensor_mul(
                                    outT[h * 64:(h + 1) * 64, hp,
                                         qc * 512:(qc + 1) * 512],
                                    asbs[h][0:HD, :],
                                    rb[:],
                                )

                        return ctx, emit_s_exp, emit_exp, emit_av, finish

                    av_queue = []   # lagging AV / finish closures
                    chunks = [(hp, qc) for hp in range(NHP) for qc in range(2)]
                    pendings = {}   # hp -> interleave steps
                    for hp in range(NHP):
                        steps = []
                        if hp + 1 < NHP:
                            steps = qk_steps_for(hp + 1) + qk_steps_for(
                                NHP + hp + 1)
                        pendings[hp] = steps

                    for ci, (hp, qc) in enumerate(chunks):
                        if qc == 0:
                            # W_proj k-tile for this head pair (ScalarE queue)
                            wpst = wpstg.tile([128, D], f32, tag="wpst",
                                              name="wpst")
                            nc.scalar.dma_start(
                                wpst[:], wproj[hp * 128:(hp + 1) * 128, :]
                            )
                            nc.vector.tensor_copy(wp_sb[:, hp, :], wpst[:])
                        ctx, emit_s_exp, emit_exp, emit_av, finish = \
                            make_chunk(hp, qc)
                        for kt2 in range(0, NT, 2):
                            if hp == NHP - 1 and qc == 1 and kt2 == 2:
                                # outT for tokens 0..511 is complete once the
                                # lagging finish(hp5, qc0) has been emitted
                                # (first pop of this chunk): project them here
                                for tt_ in range(4):
                                    pendings[hp] += proj_steps_for(tt_)
                            ssa = emit_s_exp(kt2)
                            ssb = emit_s_exp(kt2 + 1)
                            emit_exp(ssa)
                            emit_exp(ssb)
                            # lagging AV work (possibly from the previous
                            # chunk, including its normalize)
                            for _ in range(2):
                                if av_queue:
                                    av_queue.pop(0)()
                            av_queue.append(
                                (lambda k=kt2, f=emit_av: f(k)))
                            av_queue.append(
                                (lambda k=kt2 + 1, f=emit_av: f(k)))
                            if kt2 == NT - 2:
                                av_queue.append(finish)
                            # interleave steps for this head pair
                            for _ in range(4 if hp == NHP - 1 else 2):
                                if pendings[hp]:
                                    pendings[hp].pop(0)()
                    # drain
                    for cl in av_queue:
                        cl()
                    for hp in range(NHP):
                        for step in pendings[hp]:
                            step()

            # ---- output projection tail (token tiles 4..7) on a dedicated
            #      4-deep PSUM rotation (attention PSUM pools are closed) ----
            with tc.tile_pool(name="tail_ps", bufs=4, space="PSUM") as tail_ps:
                for tt in range(4, NT):
                    ps0 = tail_ps.tile([128, 384], f32, tag="ty0", name="ty0")
                    ps1 = tail_ps.tile([128, 384], f32, tag="ty1", name="ty1")
                    halves = (ps0, ps1)
                    for j in range(NHP):
                        for fc in range(2):
                            nc.tensor.matmul(
                                halves[fc][:],
                                outT[:, j, tt * 128:(tt + 1) * 128],
                                wp_sb[:, j, fc * 384:(fc + 1) * 384],
                                start=(j == 0),
                                stop=(j == NHP - 1),
                            )
                    for fc in range(2):
                        # fused evict + bias add
                        yst = ystage.tile([128, 384], f32, tag="yst",
                                          name="yst")
                        nc.vector.tensor_add(
                            yst[:], halves[fc][:],
                            bias_bc[:, fc * 384:(fc + 1) * 384],
                        )
                        nc.sync.dma_start(
                            y[tt * 128:(tt + 1) * 128,
                              fc * 384:(fc + 1) * 384],
                            yst[:],
                        )

    nc.compile()
    return nc


def kernel(**inputs) -> np.ndarray:
    from concourse.bass_utils import run_bass_kernel_spmd

    x = np.ascontiguousarray(np.asarray(inputs["x"], dtype=np.float32))
    wqkv = np.ascontiguousarray(np.asarray(inputs["W_qkv"], dtype=np.float32))
    wproj = np.ascontiguousarray(np.asarray(inputs["W_proj"], dtype=np.float32))
    bproj = np.ascontiguousarray(np.asarray(inputs["b_proj"], dtype=np.float32))

    if "nc" not in _STATE:
        _STATE["nc"] = _build()
    nc = _STATE["nc"]

    in_maps = [
        {"x": x[b], "w_qkv": wqkv, "w_proj": wproj, "b_proj": bproj}
        for b in range(B)
    ]
    res = run_bass_kernel_spmd(nc, in_maps, list(range(B)))
    out = np.stack([res.results[b]["y"] for b in range(B)], axis=0)
    return out.astype(np.float32)


# revision 26
# speedup vs baseline: 1.0760x; 1.0232x over previous
"""Multi-head attention block (B=8, N=1024, D=768, H=12 heads) on 8 trn2 NeuronCores.

Sharding: pure data-parallel over the batch dimension (one batch element per
core, weights replicated). No collectives needed.

Per-core kernel (Bass/Tile). v5:
  - all matmul operands bf16 (psum f32): FWL on stationary loads, 16-bit DVE
  - PE transposes (normal-mode matmul vs bf16 identity) with merged ScalarE
    evictions; x staged 8-deep on its own DMA queue, weights on the ScalarE
    queue
  - row-tiled score matmuls batched in kt-pairs (fewer tiling transitions)
  - attention flattened into one software-pipelined stream: AV matmuls lag S/exp
    by 2 k-tiles ACROSS (hp,qc) chunk boundaries so ScalarE never drains
  - softmax denominator broadcast via gpsimd.partition_broadcast (no DRAM
    round-trip)
  - output-projection tail on a dedicated 4-deep PSUM rotation
"""

import numpy as np

B, N, D = 8, 1024, 768
NH, HD = 12, 64
SCALE = HD ** -0.5  # 0.125
NT = N // 128       # 8 token tiles
NKT = D // 128      # 6 contraction tiles over D
NHP = NH // 2       # 6 head pairs

_STATE = {}


def _build():
    import concourse.bacc as bacc
    import concourse.bass as bass
    import concourse.mybir as mybir
    from concourse import tile
    from concourse.masks import make_identity

    f32 = mybir.dt.float32
    bf16 = mybir.dt.bfloat16
    EXP = mybir.ActivationFunctionType.Exp

    nc = bacc.Bacc(None, target_bir_lowering=False)
    x = nc.dram_tensor("x", [N, D], f32, kind="ExternalInput")
    wqkv = nc.dram_tensor("w_qkv", [D, 3 * D], f32, kind="ExternalInput")
    wproj = nc.dram_tensor("w_proj", [D, D], f32, kind="ExternalInput")
    bproj = nc.dram_tensor("b_proj", [D], f32, kind="ExternalInput")
    y = nc.dram_tensor("y", [N, D], f32, kind="ExternalOutput")

    with tile.TileContext(nc) as tc:
        with (
            tc.tile_pool(name="const", bufs=1) as const,
            tc.tile_pool(name="big", bufs=1) as big,
            tc.tile_pool(name="ystage", bufs=3) as ystage,
        ):
            ident = const.tile([128, 128], bf16)
            make_identity(nc, ident[:])
            zb = const.tile([128, 1], f32)
            nc.vector.memset(zb[:], 0.0)
            onef = const.tile([128, 1], f32)
            nc.vector.memset(onef[:], 1.0)
            # b_proj broadcast across partitions: tiny DMA to partition 0,
            # then an on-chip partition broadcast (keeps the slow zero-stride
            # DMA replication off the HBM-bound prologue window)
            bias_row = const.tile([1, D], f32)
            nc.sync.dma_start(bias_row[:], bproj[None, :])
            bias_bc = const.tile([128, D], f32)
            nc.gpsimd.partition_broadcast(bias_bc[:], bias_row[:])

            # persistent activations (all bf16)
            qkT = big.tile([128, 2 * NHP, N], bf16)       # q ftiles 0..5, k 6..11
            vban = big.tile([128, NT, NH, HD + 1], bf16)  # v natural + ones col
            outT = big.tile([128, NHP, N], bf16)          # attention out, transposed
            xT = big.tile([128, NKT, N], bf16)            # x transposed
            wv_sb = big.tile([128, NKT, D], bf16)
            wp_sb = big.tile([128, NKT, D], bf16)

            # ones columns for the denominator trick (value cols written by the
            # v-eviction copies below)
            nc.vector.tensor_copy(
                vban[:, :, :, HD:HD + 1].rearrange("p a b one -> p (a b one)"),
                onef[:, 0:1].to_broadcast((128, NT * NH)),
            )

            with (
                tc.tile_pool(name="s_ps", bufs=2, space="PSUM") as s_ps,
                tc.tile_pool(name="acc_ps", bufs=1, space="PSUM") as acc_ps,
                tc.tile_pool(name="qk_ps", bufs=1, space="PSUM") as qk_ps,
            ):
                # ---- Phase 1: transposes (PE) + v projection + q/k prologue
                #      as one dense PE stretch. x tiles stream on the Sync
                #      DMA queue; weights on the ScalarE DMA queue. ----
                with (
                    tc.tile_pool(name="stage", bufs=4) as stage,
                    tc.tile_pool(name="wstg", bufs=3) as wstg,
                ):
                    # one priority-ordered stream on the Sync queue: x tiles
                    # (PE-critical) interleaved with the v-weights, then the
                    # q/k-prologue weight tiles
                    xsts, wvsts, wtsts = [], [], []

                    def load_x(tt):
                        xst = stage.tile([128, D], f32, tag="xst",
                                         name="xst")
                        nc.sync.dma_start(xst[:], x[tt * 128:(tt + 1) * 128, :])
                        xsts.append(xst)

                    def load_wv(kt):
                        wvst = wstg.tile([128, D], f32, tag="wvst",
                                         name="wvst")
                        nc.sync.dma_start(
                            wvst[:], wqkv[kt * 128:(kt + 1) * 128, 2 * D:3 * D]
                        )
                        wvsts.append(wvst)

                    for tt in range(3):
                        load_x(tt)
                    for kt in range(NKT):
                        load_wv(kt)
                        if kt + 3 < NT:
                            load_x(kt + 3)
                    for i, ft in enumerate([0] * NKT + [NHP] * NKT):
                        kt = i % NKT
                        wt = wstg.tile([128, 128], f32, tag="wt",
                                       name="wt")
                        nc.sync.dma_start(
                            wt[:],
                            wqkv[kt * 128:(kt + 1) * 128,
                                 ft * 128:(ft + 1) * 128],
                        )
                        wtsts.append(wt)
                    xnbs = []
                    for tt in range(NT):
                        xnb = stage.tile([128, D], bf16, tag="xnb",
                                         name="xnb")
                        nc.vector.tensor_copy(xnb[:], xsts[tt][:])
                        xnbs.append(xnb)
                        if tt < NKT:
                            nc.vector.tensor_copy(wv_sb[:, tt, :],
                                                  wvsts[tt][:])

                    def emit_v(tt):
                        psv = s_ps.tile([128, 1024], f32, tag="s", name="psv")
                        halves = (psv[:, 0:384], psv[:, 512:896])
                        for kt in range(NKT):
                            for fc in range(2):
                                nc.tensor.matmul(
                                    halves[fc],
                                    xT[:, kt, tt * 128:(tt + 1) * 128],
                                    wv_sb[:, kt, fc * 384:(fc + 1) * 384],
                                    start=(kt == 0),
                                    stop=(kt == NKT - 1),
                                )
                        for fc in range(2):
                            nc.vector.tensor_copy(
                                vban[:, tt, fc * 6:(fc + 1) * 6, 0:HD],
                                halves[fc].rearrange("p (h d) -> p h d", h=6),
                            )

                    # transpose x on the PE (normal-mode matmul against the
                    # bf16 identity); evictions are 2 wide ScalarE copies per
                    # token tile; PSUM banks alternate between the (idle) qk
                    # and acc rings so eviction overlaps the next transposes
                    for tt in range(NT):
                        pool = qk_ps if tt % 2 == 0 else acc_ps
                        tags = ("psq0", "psq1") if tt % 2 == 0 else \
                               ("acc0", "acc1")
                        pa = pool.tile([128, 512], f32, tag=tags[0], name="xpa")
                        pb = pool.tile([128, 512], f32, tag=tags[1], name="xpb")
                        for dt_ in range(NKT):
                            dst = (pa[:, dt_ * 128:(dt_ + 1) * 128] if dt_ < 4
                                   else pb[:, (dt_ - 4) * 128:(dt_ - 3) * 128])
                            nc.tensor.matmul(
                                dst,
                                xnbs[tt][:, dt_ * 128:(dt_ + 1) * 128],
                                ident[:],
                                start=True,
                                stop=True,
                            )
                        nc.scalar.copy(
                            xT[:, 0:4, tt * 128:(tt + 1) * 128],
                            pa[:].rearrange("p (a b) -> p a b", a=4),
                        )
                        nc.scalar.copy(
                            xT[:, 4:6, tt * 128:(tt + 1) * 128],
                            pb[:, 0:256].rearrange("p (a b) -> p a b", a=2),
                        )
                        if tt >= 1:
                            emit_v(tt - 1)
                    emit_v(NT - 1)

                    # q/k prologue for head pair 0: ft 0 (q) in the qk_ps
                    # banks, ft 6 (k) in the acc_ps banks; evictions on the
                    # (mostly idle) ScalarE
                    def qk_prologue_steps(ft, pool, tags, base):
                        psq0 = pool.tile([128, 512], f32, tag=tags[0],
                                         name=f"pq{ft}a")
                        psq1 = pool.tile([128, 512], f32, tag=tags[1],
                                         name=f"pq{ft}b")
                        psqs = (psq0, psq1)
                        for kt in range(NKT):
                            wtb = wstg.tile([128, 128], bf16, tag="wtb0",
                                            name="wtb0")
                            nc.vector.tensor_copy(wtb[:], wtsts[base + kt][:])
                            for qch in range(2):
                                nc.tensor.matmul(
                                    psqs[qch][:],
                                    wtb[:],
                                    xT[:, kt, qch * 512:(qch + 1) * 512],
                                    start=(kt == 0),
                                    stop=(kt == NKT - 1),
                                )
                        for qch in range(2):
                            nc.scalar.copy(
                                qkT[:, ft, qch * 512:(qch + 1) * 512],
                                psqs[qch][:],
                            )

                    qk_prologue_steps(0, qk_ps, ("psq0", "psq1"), 0)
                    qk_prologue_steps(NHP, acc_ps, ("acc0", "acc1"), NKT)

                # ---- attention: one flat software-pipelined stream over the
                #      12 (hp, qc) chunks. Per kt-pair superslot: 2 row-tiled
                #      S pairs, 2 exps, then 2 AV steps popped from a lagging
                #      queue (which crosses chunk boundaries), then 2-4
                #      interleave steps (q/k projection for the next head
                #      pair; output projection of finished token tiles). ----
                with (
                    tc.tile_pool(name="wq_pool", bufs=8) as wq_pool,
                    tc.tile_pool(name="wqb_pool", bufs=8) as wqb_pool,
                    tc.tile_pool(name="pt_pool", bufs=6) as pt_pool,
                    tc.tile_pool(name="wpstg", bufs=2) as wpstg,
                    tc.tile_pool(name="norm", bufs=2) as norm,
                ):
                    def qk_steps_for(ft):
                        """Generator of closures; each emits one PE step of the
                        qT/kT projection for feature tile ft (2 psum halves).
                        PSUM is allocated lazily inside the first step."""
                        box = {}

                        def mk_mm(kt):
                            def emit():
                                if "psqs" not in box:
                                    box["psqs"] = (
                                        qk_ps.tile([128, 512], f32, tag="psq0",
                                                   name="psq0"),
                                        qk_ps.tile([128, 512], f32, tag="psq1",
                                                   name="psq1"),
                                    )
                                wt = wq_pool.tile(
                                    [128, 128], f32, tag="wt", name="wt"
                                )
                                nc.sync.dma_start(
                                    wt[:],
                                    wqkv[kt * 128:(kt + 1) * 128,
                                         ft * 128:(ft + 1) * 128],
                                )
                                wtb = wqb_pool.tile(
                                    [128, 128], bf16, tag="wtb", name="wtb"
                                )
                                nc.vector.tensor_copy(wtb[:], wt[:])
                                for qch in range(2):
                                    nc.tensor.matmul(
                                        box["psqs"][qch][:],
                                        wtb[:],
                                        xT[:, kt, qch * 512:(qch + 1) * 512],
                                        start=(kt == 0),
                                        stop=(kt == NKT - 1),
                                    )
                            return emit

                        def mk_evict():
                            def emit():
                                for qch in range(2):
                                    nc.vector.tensor_copy(
                                        qkT[:, ft, qch * 512:(qch + 1) * 512],
                                        box["psqs"][qch][:],
                                    )
                            return emit

                        return [mk_mm(kt) for kt in range(NKT)] + [mk_evict()]

                    def proj_steps_for(tt):
                        """Output projection of token tile tt as interleavable
                        steps (borrows the idle qk PSUM slots)."""
                        box = {}

                        def mk_mm(j0):
                            def emit():
                                if "psys" not in box:
                                    box["psys"] = (
                                        qk_ps.tile([128, 512], f32, tag="psq0",
                                                   name="psy0"),
                                        qk_ps.tile([128, 512], f32, tag="psq1",
                                                   name="psy1"),
                                    )
                                for j in (j0, j0 + 1):
                                    for fc in range(2):
                                        nc.tensor.matmul(
                                            box["psys"][fc][:, 0:384],
                                            outT[:, j, tt * 128:(tt + 1) * 128],
                                            wp_sb[:, j, fc * 384:(fc + 1) * 384],
                                            start=(j == 0),
                                            stop=(j == NHP - 1),
                                        )
                            return emit

                        def mk_evict():
                            def emit():
                                for fc in range(2):
                                    yst = ystage.tile([128, 384], f32, tag="yst",
                                                      name="yst")
                                    nc.vector.tensor_add(
                                        yst[:], box["psys"][fc][:, 0:384],
                                        bias_bc[:, fc * 384:(fc + 1) * 384],
                                    )
                                    nc.sync.dma_start(
                                        y[tt * 128:(tt + 1) * 128,
                                          fc * 384:(fc + 1) * 384],
                                        yst[:],
                                    )
                            return emit

                        return [mk_mm(j0) for j0 in range(0, NHP, 2)] + [mk_evict()]

                    def make_chunk(hp, qc):
                        """State + closures for one (hp, qc) attention chunk."""
                        ctx = {"pts": []}

                        def emit_av(kt):
                            if "accs" not in ctx:
                                ctx["accs"] = (
                                    acc_ps.tile([HD + 1, 512], f32, tag="acc0",
                                                name="acc0"),
                                    acc_ps.tile([HD + 1, 512], f32, tag="acc1",
                                                name="acc1"),
                                )
                            pt = ctx["pts"][kt]
                            for h in range(2):
                                nc.tensor.matmul(
                                    ctx["accs"][h][:],
                                    vban[:, kt, hp * 2 + h, :],
                                    pt[:, h * 512:(h + 1) * 512],
                                    start=(kt == 0),
                                    stop=(kt == NT - 1),
                                )

                        def emit_s_exp(kt):
                            ssum = s_ps.tile([128, 1024], f32, tag="s",
                                             name="ssum")
                            for h in range(2):
                                ksl = qkT[h * 64:(h + 1) * 64, NHP + hp,
                                          kt * 128:(kt + 1) * 128]
                                qsl = qkT[h * 64:(h + 1) * 64, hp,
                                          qc * 512:(qc + 1) * 512]
                                nc.tensor.matmul(
                                    ssum[:, h * 512:(h + 1) * 512],
                                    ksl,
                                    qsl,
                                    start=True,
                                    stop=True,
                                )
                            return ssum

                        def emit_exp(ssum):
                            pt = pt_pool.tile([128, 1024], bf16, tag="pt",
                                              name="pt")
                            nc.scalar.activation(
                                pt[:], ssum[:], EXP, bias=zb[:], scale=SCALE
                            )
                            ctx["pts"].append(pt)

                        def finish():
                            # evict accumulators to SBUF (frees the acc ring),
                            # then normalize entirely from SBUF
                            accs = ctx["accs"]
                            asbs = (
                                norm.tile([HD + 1, 512], f32, tag="asb0",
                                          name="asb0"),
                                norm.tile([HD + 1, 512], f32, tag="asb1",
                                          name="asb1"),
                            )
                            nc.vector.tensor_copy(asbs[0][:], accs[0][:])
                            nc.vector.tensor_copy(asbs[1][:], accs[1][:])
                            # denominator rows -> partition 0 via SBUF->SBUF
                            # DMA (custom DVE recip needs base_partition 0)
                            dd = norm.tile([1, N], f32, tag="dd", name="dd")
                            for h in range(2):
                                nc.sync.dma_start(
                                    dd[0:1, h * 512:(h + 1) * 512],
                                    asbs[h][HD:HD + 1, :],
                                )
                            rr = norm.tile([1, N], f32, tag="rr", name="rr")
                            nc.vector.reciprocal_approx_fast(rr[:], dd[:])
                            for h in range(2):
                                rb = norm.tile([HD, 512], f32, tag=f"rb{h}",
                                               name=f"rb{h}")
                                nc.gpsimd.partition_broadcast(
                                    rb[:], rr[0:1, h * 512:(h + 1) * 512],
                                )
                                nc.vector.tensor_mul(
                                    outT[h * 64:(h + 1) * 64, hp,
                                         qc * 512:(qc + 1) * 512],
                                    asbs[h][0:HD, :],
                                    rb[:],
                                )

                        return ctx, emit_s_exp, emit_exp, emit_av, finish

                    av_queue = []   # lagging AV / finish closures
                    chunks = [(hp, qc) for hp in range(NHP) for qc in range(2)]
                    pendings = {}   # hp -> interleave steps
                    for hp in range(NHP):
                        steps = []
                        if hp + 1 < NHP:
                            steps = qk_steps_for(hp + 1) + qk_steps_for(
                                NHP + hp + 1)
                        pendings[hp] = steps

                    for ci, (hp, qc) in enumerate(chunks):
                        if qc == 0:
                            # W_proj k-tile for this head pair (ScalarE queue)
                            wpst = wpstg.tile([128, D], f32, tag="wpst",
                                              name="wpst")
                            nc.scalar.dma_start(
                                wpst[:], wproj[hp * 128:(hp + 1) * 128, :]
                            )
                            nc.vector.tensor_copy(wp_sb[:, hp, :], wpst[:])
                        ctx, emit_s_exp, emit_exp, emit_av, finish = \
                            make_chunk(hp, qc)
                        for kt2 in range(0, NT, 2):
                            if hp == NHP - 1 and qc == 1 and kt2 == 2:
                                # outT for tokens 0..511 is complete once the
                                # lagging finish(hp5, qc0) has been emitted
                                # (first pop of this chunk): project them here
                                for tt_ in range(4):
                                    pendings[hp] += proj_steps_for(tt_)
                            ssa = emit_s_exp(kt2)
                            ssb = emit_s_exp(kt2 + 1)
                            emit_exp(ssa)
                            emit_exp(ssb)
                            # lagging AV work (possibly from the previous
                            # chunk, including its normalize)
                            for _ in range(2):
                                if av_queue:
                                    av_queue.pop(0)()
                            av_queue.append(
                                (lambda k=kt2, f=emit_av: f(k)))
                            av_queue.append(
                                (lambda k=kt2 + 1, f=emit_av: f(k)))
                            if kt2 == NT - 2:
                                av_queue.append(finish)
                            # interleave steps for this head pair
                            for _ in range(4 if hp == NHP - 1 else 2):
                                if pendings[hp]:
                                    pendings[hp].pop(0)()
                    # drain
                    for cl in av_queue:
                        cl()
                    for hp in range(NHP):
                        for step in pendings[hp]:
                            step()

            # ---- output projection tail (token tiles 4..7) on a dedicated
            #      4-deep PSUM rotation (attention PSUM pools are closed) ----
            with tc.tile_pool(name="tail_ps", bufs=4, space="PSUM") as tail_ps:
                for tt in range(4, NT):
                    ps0 = tail_ps.tile([128, 384], f32, tag="ty0", name="ty0")
                    ps1 = tail_ps.tile([128, 384], f32, tag="ty1", name="ty1")
                    halves = (ps0, ps1)
                    for j in range(NHP):
                        for fc in range(2):
                            nc.tensor.matmul(
                                halves[fc][:],
                                outT[:, j, tt * 128:(tt + 1) * 128],
                                wp_sb[:, j, fc * 384:(fc + 1) * 384],
                                start=(j == 0),
                                stop=(j == NHP - 1),
                            )
                    for fc in range(2):
                        # fused evict + bias add
                        yst = ystage.tile([128, 384], f32, tag="yst",
                                          name="yst")
                        nc.vector.tensor_add(
                            yst[:], halves[fc][:],
                            bias_bc[:, fc * 384:(fc + 1) * 384],
                        )
                        nc.sync.dma_start(
                            y[tt * 128:(tt + 1) * 128,
                              fc * 384:(fc + 1) * 384],
                            yst[:],
                        )

    nc.compile()
    return nc


def kernel(**inputs) -> np.ndarray:
    from concourse.bass_utils import run_bass_kernel_spmd

    x = np.ascontiguousarray(np.asarray(inputs["x"], dtype=np.float32))
    wqkv = np.ascontiguousarray(np.asarray(inputs["W_qkv"], dtype=np.float32))
    wproj = np.ascontiguousarray(np.asarray(inputs["W_proj"], dtype=np.float32))
    bproj = np.ascontiguousarray(np.asarray(inputs["b_proj"], dtype=np.float32))

    if "nc" not in _STATE:
        _STATE["nc"] = _build()
    nc = _STATE["nc"]

    in_maps = [
        {"x": x[b], "w_qkv": wqkv, "w_proj": wproj, "b_proj": bproj}
        for b in range(B)
    ]
    res = run_bass_kernel_spmd(nc, in_maps, list(range(B)))
    out = np.stack([res.results[b]["y"] for b in range(B)], axis=0)
    return out.astype(np.float32)
